# revision 1
# baseline (speedup 1.0000x reference)
"""Trainium2 Bass kernel for nn_CrossAttentionFusion.

The reference module is a cross-attention with seq_len==1 on both sides:
softmax over a single key is identically 1, so the Q/K projections are
dead code and the whole module collapses to

    y[b,0] = LN(x[b,0] + x[b,1] @ Weff.T + beff)
    y[b,1] = LN(x[b,1] + x[b,0] @ Weff.T + beff)

with Weff = Wo @ Wv, beff = Wo @ bv + bo.  This is a memory-bound
[1M x 256] x [256 x 256] matmul + residual + LayerNorm.

Distribution: pure data parallel over batch across 8 NeuronCores.

Per-core kernel layout (all fp32 in DRAM):
  - x viewed as rows [(b, plane), 256] -> tiles [128 rows, 512] where the
    free dim holds both planes of 128 batch elements (2 KiB contiguous per
    partition -> full-bandwidth DMA).
  - PE transposes the four 128x128 blocks into PSUM; DVE+ACT copy them to
    SBUF as float32r (TF32-class rounding, full-rate fp32 matmul).
  - PE computes y = x + xswap @ Weff.T directly in PSUM with four N=512
    f32r matmuls per pair-group: the moving operand [Weff.T_c | I_c]
    produces the fused product for one plane and the other plane's
    residual in one pass; output lands natural-orientation, so no
    transpose-back and no DVE/ACT adds.
  - DVE bn_stats/bn_aggr give per-row mean/var; ACT sqrt + DVE reciprocal
    give rstd; ACT applies (y - mean) * rstd while copying PSUM -> SBUF.
  - Non-trivial beff / ln_gamma / ln_beta are handled by optional extra
    instructions, emitted only when the runtime values need them.
"""

import sys

import numpy as np

sys.path.insert(0, "/opt/trn_rl_repo")

import concourse.bass as bass  # noqa: E402
import concourse.tile as tile  # noqa: E402
from concourse import bass_utils, mybir  # noqa: E402

B = 524288
D = 256
N_CORES = 8
LN_EPS = 1e-5

_F32 = mybir.dt.float32
_F32R = mybir.dt.float32r


def split_waits(nc, limit=1):
    """Hoist excess sync waits onto single-wait EventSemaphore instructions.

    The walrus in this toolchain rejects instructions carrying more than one
    sync wait ("Too many sync wait commands"), so we post-process the BIR:
    for any instruction with >limit waits, emit preceding EventSemaphore
    instructions (same engine, program order) each carrying one wait.
    """
    n_new = 0
    for f in nc.m.functions:
        for bb in f.blocks:
            out = []
            for inst in bb.instructions:
                si = getattr(inst, "sync_info", None)
                ow = list(si.on_wait) if (si is not None and si.on_wait) else []
                if len(ow) > limit:
                    for k, w in enumerate(ow[:-limit]):
                        es = mybir.InstEventSemaphore(
                            name=f"{inst.name}_ws{k}",
                            engine=inst.engine,
                            ins=[], outs=[],
                            sync_info=mybir.SyncInfo(on_wait=[w], on_update=[]),
                        )
                        nc.inst_map[es.name] = es
                        out.append(es)
                        n_new += 1
                    inst.sync_info = mybir.SyncInfo(
                        on_wait=ow[-limit:], on_update=list(si.on_update or []))
                out.append(inst)
            bb.instructions[:] = out
    return n_new


def build_nc(n, G=4, with_bias=False, with_gamma=False, with_beta=False,
             use_f32r=True, repeats=1, bufs=None, skip=(),
             split_rings=False, pg_layout=True):
    """Build the per-core Bass program for n batch elements (2*n rows).

    repeats>1 wraps the whole body in a For_i that redoes the identical
    work (idempotent) — used only to amplify kernel time over the ~79 ms
    axon dispatch jitter when benchmarking.
    """
    assert n % (128 * G) == 0, (n, G)
    n_mega = n // (128 * G)
    bf = {"xin": 3, "xt": 6, "yout": 3, "stat": 8, "tps": 3, "yps": 3}
    if bufs:
        bf.update(bufs)

    # "(p g)" maps each partition to one contiguous G*2KiB DRAM block per
    # megatile DMA (vs G strided 2KiB chunks for "(g p)") — which batch
    # elements share a pair-group is irrelevant to the math.
    rowmap = "(p g) f -> p g f" if pg_layout else "(g p) f -> p g f"

    nc = bass.Bass(trn_type="TRN2")
    x_d = nc.dram_tensor("x", [n, 2 * D], _F32, kind="ExternalInput")
    # wcat[p][c] = moving operand for input plane p, contraction chunk c:
    #   p=1: [Weff.T_c | I_c]  (cols 0:D -> y0 fused, D:2D -> y1 residual)
    #   p=0: [I_c | Weff.T_c]  (cols 0:D -> y0 residual, D:2D -> y1 fused)
    wcat_d = nc.dram_tensor("wcat", [2, 2, 128, 2 * D], _F32,
                            kind="ExternalInput")
    id_d = nc.dram_tensor("ident", [128, 128], _F32, kind="ExternalInput")
    if with_bias:
        beff_d = nc.dram_tensor("beff", [1, D], _F32, kind="ExternalInput")
    if with_gamma:
        gamma_d = nc.dram_tensor("gamma", [1, D], _F32, kind="ExternalInput")
    if with_beta:
        beta_d = nc.dram_tensor("beta", [1, D], _F32, kind="ExternalInput")
    out_d = nc.dram_tensor("out", [n, 2 * D], _F32, kind="ExternalOutput")

    mm_dt = _F32R if use_f32r else _F32

    with tile.TileContext(nc) as tc:
        with (
            tc.tile_pool(name="const", bufs=1) as constp,
            tc.tile_pool(name="xin", bufs=bf["xin"]) as xinp,
            tc.tile_pool(name="xt", bufs=bf["xt"]) as xtp,
            tc.tile_pool(name="yout", bufs=bf["yout"]) as youtp,
            tc.tile_pool(name="stat", bufs=bf["stat"]) as statp,
            tc.tile_pool(name="tps", bufs=bf["tps"], space="PSUM") as tpsum,
            tc.tile_pool(name="yps", bufs=bf["yps"], space="PSUM") as ypsum,
        ):
            wcat_sb = constp.tile([128, 2, 2, 2 * D], _F32)
            nc.sync.dma_start(
                out=wcat_sb, in_=wcat_d[:].rearrange("p c q f -> q p c f"))
            id_sb = constp.tile([128, 128], _F32)
            nc.sync.dma_start(out=id_sb, in_=id_d[:])
            eps_sb = constp.tile([128, 1], _F32)
            nc.vector.memset(eps_sb, LN_EPS)
            if with_bias:
                ones_sb = constp.tile([1, 128], _F32)
                nc.vector.memset(ones_sb, 1.0)
                beff_sb = constp.tile([1, 2 * D], _F32)
                nc.sync.dma_start(
                    out=beff_sb,
                    in_=bass.AP(tensor=beff_d[:].tensor, offset=0,
                                ap=[[0, 1], [0, 2], [1, D]]),
                )
            if with_gamma:
                gamma_sb = constp.tile([128, D], _F32)
                nc.gpsimd.dma_start(
                    out=gamma_sb,
                    in_=bass.AP(tensor=gamma_d[:].tensor, offset=0,
                                ap=[[0, 128], [1, D]]),
                )
            if with_beta:
                beta_sb = constp.tile([128, D], _F32)
                nc.gpsimd.dma_start(
                    out=beta_sb,
                    in_=bass.AP(tensor=beta_d[:].tensor, offset=0,
                                ap=[[0, 128], [1, D]]),
                )

            if use_f32r:
                wcat_mm = constp.tile([128, 2, 2, 2 * D], _F32R)
                nc.vector.tensor_copy(out=wcat_mm, in_=wcat_sb)
            else:
                wcat_mm = wcat_sb

            def megatile(m):
                r0 = m * G * 128
                xin = xinp.tile([128, G, 2 * D], _F32)
                nc.sync.dma_start(
                    out=xin,
                    in_=x_d[r0:r0 + G * 128, :].rearrange(rowmap, p=128),
                )
                yo = youtp.tile([128, G, 2 * D], _F32)
                for j in range(G):
                    xj = xin[:, j, :]  # [128, 512]: x0 | x1
                    # ---- transpose the four 128x128 blocks into PSUM ----
                    tp = tpsum.tile([128, 4, 128], _F32)
                    for c in range(1 if "t1" in skip else 4):
                        nc.tensor.matmul(
                            out=tp[:, c, :],
                            lhsT=xj[:, c * 128:(c + 1) * 128],
                            rhs=id_sb,
                            is_transpose=True, start=True, stop=True,
                        )
                    # ---- copy PSUM -> SBUF (split across DVE and ACT) ----
                    xt = xtp.tile([128, 4, 128], mm_dt)
                    nc.vector.tensor_copy(out=xt[:, 0:1, :], in_=tp[:, 0:1, :])
                    nc.scalar.copy(out=xt[:, 1:4, :], in_=tp[:, 1:4, :])

                    # ---- y = x + xswap @ Weff.T in PSUM ----
                    # Each matmul covers the full [128, 512] yp tile:
                    # rhs [Weff.T_c|I_c] places the fused product and the
                    # other plane's residual in the right column halves.
                    yp = ypsum.tile([128, 2, D], _F32)
                    mms = ([(p, c) for p in range(2) for c in range(2)]
                           if "mm" not in skip else [(0, 0)])
                    for i, (p, c) in enumerate(mms):
                        nc.tensor.matmul(
                            out=yp,
                            lhsT=xt[:, 2 * p + c, :],
                            rhs=wcat_mm[:, p, c, :],
                            start=(i == 0),
                            stop=(i == len(mms) - 1 and not with_bias),
                            skip_group_check=True,
                        )
                    if with_bias:
                        nc.tensor.matmul(
                            out=yp,
                            lhsT=ones_sb,
                            rhs=beff_sb,
                            start=False, stop=True, skip_group_check=True,
                        )

                    # ---- LayerNorm stats ----
                    if "stats" not in skip:
                        st = statp.tile([128, 2, 6], _F32)
                        nc.vector.bn_stats(out=st[:, 0, :], in_=yp[:, 0, :])
                        nc.vector.bn_stats(out=st[:, 1, :], in_=yp[:, 1, :])
                        mv = statp.tile([128, 2, 2], _F32)
                        nc.vector.bn_aggr(out=mv[:, 0, :], in_=st[:, 0, :])
                        nc.vector.bn_aggr(out=mv[:, 1, :], in_=st[:, 1, :])
                        rstd = statp.tile([128, 2], _F32)
                        nc.scalar.activation(
                            out=rstd, in_=mv[:, :, 1],
                            func=mybir.ActivationFunctionType.Sqrt,
                            bias=eps_sb, scale=1.0,
                        )
                        nc.vector.reciprocal(out=rstd, in_=rstd)
                        nmr = statp.tile([128, 2], _F32)
                        nc.vector.scalar_tensor_tensor(
                            out=nmr, in0=mv[:, :, 0], scalar=-1.0, in1=rstd,
                            op0=mybir.AluOpType.mult, op1=mybir.AluOpType.mult,
                        )
                    # ---- normalize: (y - mean) * rstd, PSUM -> SBUF ----
                    for h in range(2):
                        if "stats" in skip or "norm" in skip:
                            nc.scalar.copy(out=yo[:, j, h * D:(h + 1) * D],
                                           in_=yp[:, h, :])
                        else:
                            nc.scalar.activation(
                                out=yo[:, j, h * D:(h + 1) * D],
                                in_=yp[:, h, :],
                                func=mybir.ActivationFunctionType.Identity,
                                bias=nmr[:, h:h + 1], scale=rstd[:, h:h + 1],
                            )
                    if with_gamma:
                        for h in range(2):
                            nc.vector.tensor_mul(
                                out=yo[:, j, h * D:(h + 1) * D],
                                in0=yo[:, j, h * D:(h + 1) * D],
                                in1=gamma_sb,
                            )
                    if with_beta:
                        for h in range(2):
                            nc.vector.tensor_add(
                                out=yo[:, j, h * D:(h + 1) * D],
                                in0=yo[:, j, h * D:(h + 1) * D],
                                in1=beta_sb,
                            )
                store_eng = nc.scalar if split_rings else nc.sync
                store_eng.dma_start(
                    out=out_d[r0:r0 + G * 128, :].rearrange(rowmap, p=128),
                    in_=yo,
                )

            if repeats > 1:
                with tc.For_i(0, repeats, 1):
                    for m in range(n_mega):
                        megatile(m)
            else:
                for m in range(n_mega):
                    megatile(m)
    split_waits(nc)
    return nc


def _prepare(inputs):
    """Host-side prep: collapse weights, decide optional paths, shard."""
    x = np.ascontiguousarray(np.asarray(inputs["x"], dtype=np.float32))
    ipw = np.asarray(inputs["in_proj_w"], dtype=np.float32)
    ipb = np.asarray(inputs["in_proj_b"], dtype=np.float32)
    opw = np.asarray(inputs["out_proj_w"], dtype=np.float32)
    opb = np.asarray(inputs["out_proj_b"], dtype=np.float32)
    gamma = np.asarray(inputs["ln_gamma"], dtype=np.float32)
    beta = np.asarray(inputs["ln_beta"], dtype=np.float32)

    d = x.shape[2]
    wv = ipw[2 * d:3 * d]
    bv = ipb[2 * d:3 * d]
    weff_t = np.ascontiguousarray((opw @ wv).T)          # [in_f, out_f]
    beff = opw @ bv + opb                                # [out_f]

    with_bias = bool(np.any(beff != 0.0))
    with_gamma = bool(np.any(gamma != 1.0))
    with_beta = bool(np.any(beta != 0.0))

    nb = x.shape[0]
    per_core = nb // N_CORES
    xr = x.reshape(nb, 2 * d)

    eye = np.eye(d, dtype=np.float32)
    wcat = np.empty((2, 2, 128, 2 * d), dtype=np.float32)
    for c in range(2):
        rows = slice(c * 128, (c + 1) * 128)
        wcat[1, c] = np.concatenate([weff_t[rows], eye[rows]], axis=1)
        wcat[0, c] = np.concatenate([eye[rows], weff_t[rows]], axis=1)
    base = {
        "wcat": wcat,
        "ident": np.eye(128, dtype=np.float32),
    }
    if with_bias:
        base["beff"] = np.ascontiguousarray(beff.reshape(1, d))
    if with_gamma:
        base["gamma"] = np.ascontiguousarray(gamma.reshape(1, d))
    if with_beta:
        base["beta"] = np.ascontiguousarray(beta.reshape(1, d))

    in_maps = []
    for c in range(N_CORES):
        m = dict(base)
        m["x"] = xr[c * per_core:(c + 1) * per_core]
        in_maps.append(m)
    return in_maps, per_core, (with_bias, with_gamma, with_beta), x.shape


def kernel(x, in_proj_w, in_proj_b, out_proj_w, out_proj_b, ln_gamma, ln_beta,
           _trace=False, _G=4):
    inputs = dict(x=x, in_proj_w=in_proj_w, in_proj_b=in_proj_b,
                  out_proj_w=out_proj_w, out_proj_b=out_proj_b,
                  ln_gamma=ln_gamma, ln_beta=ln_beta)
    in_maps, per_core, (wb, wg, wbt), xshape = _prepare(inputs)
    nc = build_nc(per_core, G=_G, with_bias=wb, with_gamma=wg, with_beta=wbt)
    res = bass_utils.run_bass_kernel_spmd(
        nc, in_maps, core_ids=list(range(N_CORES)), trace=_trace,
    )
    out = np.concatenate([r["out"] for r in res.results], axis=0)
    kernel.last_results = res
    return out.reshape(xshape)

